# revision 1
# baseline (speedup 1.0000x reference)
"""Trainium2 Bass kernel for CorrelatedGraphConv.

Reference computation (per batch b, N=100 rows, D=1024, L=2000 labels):
    adj   = (graph != 0)
    lin   = x + x@W0.T + x@W1.T + sum_j bias[graph[:, j]]
    a     = x@Wa.T + ba ; bvec = x@Wb.T + bb
    alpha = relu(a @ bvec.T)
    alpha = softmax(adj @ alpha, axis=0)   # over rows i
    out   = alpha @ lin

Strategy: data-parallel over batch across 8 cores (2 batches/core).
The label-gather `sum_j bias[g[i,j]]` is computed as `C @ bias` where
C[i, l] = #{j : g[i,j] == l} is built on-chip with gpsimd.local_scatter
(per-partition indexed scatter); duplicate labels within a row are
pre-combined on DVE (count via self-equality matmul-free compare, only
the first occurrence scatters the total count).
"""

import numpy as np

import concourse.bass as bass
import concourse.mybir as mybir
import concourse.tile as tile
from concourse import bacc, library_config

F32 = mybir.dt.float32
F32R = mybir.dt.float32r
F16 = mybir.dt.float16
I16 = mybir.dt.int16

B, N, D, L = 16, 100, 1024, 2000
NCORES = 8
BPC = B // NCORES          # batches per core
R = BPC * N                # rows per core (200)
DT = D // 128              # 8 d-tiles
LT_TILES = (L + 127) // 128  # 16 label tiles (last is 80)
LPAD = 2048                  # padded label rows in scatter tables
ESC = 256                    # scatter element width (fp16) = 512B rows
NCALL = 5                    # scatter calls (one table each)
TPC = R * N // NCALL         # tokens per call (10000)
SCHUNK = (TPC + 127) // 128  # source chunks per call (79)

_CACHE = {}


def _bcast3(ap, mid, inner, mode):
    """[P, F] AP -> [P, mid, inner] broadcast view.

    mode 'j':  out[p, a, b] = ap[p, a]   (inner broadcast)
    mode 'jp': out[p, a, b] = ap[p, b]   (middle broadcast)
    """
    (pstep, pcount), (fstep, fcount) = ap.ap[0], ap.ap[1]
    if mode == "j":
        assert fcount == mid
        new = [[pstep, pcount], [fstep, mid], [0, inner]]
    else:
        assert fcount == inner
        new = [[pstep, pcount], [0, mid], [fstep, inner]]
    return bass.AP(tensor=ap.tensor, offset=ap.offset, ap=new)


def _pbcast(ap, p):
    """[1, ...] AP -> [p, ...] partition-broadcast view."""
    new = [[0, p]] + [list(d) for d in ap.ap[1:]]
    return bass.AP(tensor=ap.tensor, offset=ap.offset, ap=new)


def _build_program():
    nc = bacc.Bacc("TRN2", target_bir_lowering=False, debug=False,
                   num_devices=NCORES)

    x_d = nc.declare_dram_parameter("x", [R, D], F32, isOutput=False)
    id32_d = nc.declare_dram_parameter("id32", [128, 128], F32, isOutput=False)
    id16_d = nc.declare_dram_parameter("id16", [128, 128], F16, isOutput=False)
    g_d = nc.declare_dram_parameter("g16", [R, N], F16, isOutput=False)
    lt_d = nc.declare_dram_parameter("ltmask", [N * N], F16, isOutput=False)
    wct_d = nc.declare_dram_parameter("wct", [D, D], F32R, isOutput=False)
    wat_d = nc.declare_dram_parameter("wat", [D, D], F32R, isOutput=False)
    wbt_d = nc.declare_dram_parameter("wbt", [D, D], F32R, isOutput=False)
    bias_d = nc.declare_dram_parameter("bias", [L, D], F32R, isOutput=False)
    ba_d = nc.declare_dram_parameter("ba", [D], F32, isOutput=False)
    bb_d = nc.declare_dram_parameter("bb", [D], F32, isOutput=False)
    out_d = nc.declare_dram_parameter("out", [R, D], F32, isOutput=True)

    with tile.TileContext(nc) as tc:
        _emit(tc, x_d, id32_d, id16_d, g_d, lt_d, wct_d, wat_d, wbt_d,
              bias_d, ba_d, bb_d, out_d)
    nc.compile()
    return nc


def _emit(tc, x_d, id32_d, id16_d, g_d, lt_d, wct_d, wat_d, wbt_d, bias_d,
          ba_d, bb_d, out_d):
    nc = tc.nc
    import contextlib

    ctx = contextlib.ExitStack()
    with ctx:
        const = ctx.enter_context(tc.tile_pool(name="const", bufs=1))
        gpool = ctx.enter_context(tc.tile_pool(name="gtiles", bufs=1))
        xpool = ctx.enter_context(tc.tile_pool(name="xtiles", bufs=1))
        xt = ctx.enter_context(tc.tile_pool(name="xt", bufs=1))
        abp = ctx.enter_context(tc.tile_pool(name="abt", bufs=1))
        linp = ctx.enter_context(tc.tile_pool(name="lin", bufs=1))
        cb = ctx.enter_context(tc.tile_pool(name="cbuild", bufs=1))
        cpool = ctx.enter_context(tc.tile_pool(name="cmat", bufs=2))
        ctp = ctx.enter_context(tc.tile_pool(name="ctmat", bufs=2))
        wstream = ctx.enter_context(tc.tile_pool(name="wstream", bufs=4))
        bstream = ctx.enter_context(tc.tile_pool(name="bstream", bufs=4))
        small = ctx.enter_context(tc.tile_pool(name="small", bufs=2))
        outp = ctx.enter_context(tc.tile_pool(name="outs", bufs=2))
        pst = ctx.enter_context(tc.tile_pool(name="pst", bufs=3, space="PSUM"))
        psa = pst
        pslin = ctx.enter_context(tc.tile_pool(name="pslin", bufs=2, space="PSUM"))
        pssm = pst
        psout = pslin

        nc.gpsimd.load_library(library_config.local_scatter)

        # ---- input DMAs: x and g first (gate the PE/DVE pipelines) ----
        xg = x_d.ap()
        xb = []
        for b in range(BPC):
            t = xpool.tile([N, D], F32, tag=f"xb{b}")
            nc.sync.dma_start(out=t[:], in_=xg[b * N:(b + 1) * N, :])
            xb.append(t)
        ident32 = const.tile([128, 128], F32)
        nc.sync.dma_start(out=ident32[:], in_=id32_d.ap())
        ident16 = const.tile([128, 128], F16)
        nc.sync.dma_start(out=ident16[:], in_=id16_d.ap())
        gg = g_d.ap()
        gb = []
        for b in range(BPC):
            t = gpool.tile([N, N], F16, tag=f"gb{b}")
            nc.sync.dma_start(out=t[:], in_=gg[b * N:(b + 1) * N, :])
            gb.append(t)
        # LT mask broadcast to all partitions, split into 4 DMAs (queue spread)
        lt_sb = const.tile([128, N * N], F16)
        lt_ap = lt_d.ap()
        for q in range(4):
            nc.sync.dma_start(
                out=lt_sb[q * 32:(q + 1) * 32, :],
                in_=bass.AP(tensor=lt_ap.tensor, offset=lt_ap.offset,
                            ap=[[0, 32], [1, N * N]]),
            )
        ba_sb = const.tile([128, DT], F32)
        nc.sync.dma_start(out=ba_sb[:], in_=ba_d.ap().rearrange("(t p) -> p t", p=128))
        bb_sb = const.tile([128, DT], F32)
        nc.sync.dma_start(out=bb_sb[:], in_=bb_d.ap().rearrange("(t p) -> p t", p=128))

        # ---- X_T [din, r] via PE transpose (padded to 256 for f32r rate) ----
        RP = 256
        xt_sb = xt.tile([128, DT, RP], F32R)
        nc.vector.memset(xt_sb[:, :, R:RP].bitcast(F32), 0.0)
        for b in range(BPC):
            for dk in range(DT):
                pt = pst.tile([128, N], F32, tag="ps")
                nc.tensor.transpose(
                    out=pt[:],
                    in_=xb[b][:, dk * 128:(dk + 1) * 128],
                    identity=ident32[:N, :N],
                )
                nc.vector.tensor_copy(
                    out=xt_sb[:, dk, b * N:(b + 1) * N], in_=pt[:]
                )

        # ---- A_T / B_T (PE + ScalarE evac; weights as column panels) ----
        at_sb = abp.tile([128, DT, R], F32R, tag="at")
        bt_sb = abp.tile([128, DT, R], F32R, tag="bt")
        for w_d, bias_col, dst in (
            (wat_d, ba_sb, at_sb), (wbt_d, bb_sb, bt_sb)
        ):
            for dt_i in range(DT):
                panel = wstream.tile([128, DT, 128], F32R, tag="wpanel")
                nc.sync.dma_start(
                    out=panel[:],
                    in_=w_d.ap()[:, dt_i * 128:(dt_i + 1) * 128].rearrange(
                        "(t p) c -> p t c", p=128
                    ),
                )
                ps = psa.tile([128, RP], F32, tag="ps")
                for dk in range(DT):
                    nc.tensor.matmul(
                        out=ps[:],
                        lhsT=panel[:, dk, :],
                        rhs=xt_sb[:, dk, :],
                        start=(dk == 0),
                        stop=(dk == DT - 1),
                    )
                nc.scalar.activation(
                    out=dst[:, dt_i, :], in_=ps[:, 0:R],
                    func=mybir.ActivationFunctionType.Identity,
                    bias=bias_col[:, dt_i:dt_i + 1], scale=1.0,
                )

        # ---- LIN psums: x @ Wc.T part (counts part accumulates later) ----
        lin_ps = []
        for b in range(BPC):
            lp = pslin.tile([N, D], F32, tag="pslin")
            lin_ps.append(lp)
        for dk in range(DT):
            wt = wstream.tile([128, D], F32R, tag="wpanel")
            nc.sync.dma_start(out=wt[:], in_=wct_d.ap()[dk * 128:(dk + 1) * 128, :])
            for b in range(BPC):
                for nch in range(2):
                    sl = slice(nch * 512, (nch + 1) * 512)
                    nc.tensor.matmul(
                        out=lin_ps[b][:, sl],
                        lhsT=xt_sb[:, dk, b * N:(b + 1) * N],
                        rhs=wt[:, sl],
                        start=(dk == 0),
                        stop=False,
                    )
        bias_tiles = []
        for lc in range(LT_TILES):
            cs = min(128, L - lc * 128)
            btile = bstream.tile([128, D], F32R, tag="btile")
            nc.sync.dma_start(out=btile[:cs],
                              in_=bias_d.ap()[lc * 128:lc * 128 + cs, :])
            bias_tiles.append(btile)

        # ---- per-batch: histogram -> C^T -> counts matmul -> attention ----
        NCH = 112  # local_scatter channels covering 100 rows
        HALF = N // 2
        lt_full = lt_sb[:]
        cmats = []
        for b in range(BPC):
            gf = gb[b]
            # meq[i, j, jp] = (g[i,j] == g[i,jp])
            meq = cb.tile([NCH, N, N], F16, tag="meq")
            nc.vector.tensor_tensor(
                out=meq[:N],
                in0=_bcast3(gf[:], N, N, "j"),
                in1=_bcast3(gf[:], N, N, "jp"),
                op=mybir.AluOpType.is_equal,
            )
            # count = sum_jp meq : fold 100->50->25 (2x tensor_tensor), then reduce
            cf1 = cb.tile([NCH, N, HALF], F16, tag="cf1")
            nc.vector.tensor_tensor(
                out=cf1[:N], in0=meq[:N, :, 0:HALF], in1=meq[:N, :, HALF:N],
                op=mybir.AluOpType.add,
            )
            cf2 = cb.tile([NCH, N, HALF // 2], F16, tag="cf2")
            nc.vector.tensor_tensor(
                out=cf2[:N], in0=cf1[:N, :, 0:HALF // 2], in1=cf1[:N, :, HALF // 2:HALF],
                op=mybir.AluOpType.add,
            )
            cnt32 = cb.tile([NCH, N], F32, tag="cnt32")
            nc.vector.tensor_reduce(
                out=cnt32[:N], in_=cf2[:N], axis=mybir.AxisListType.X,
                op=mybir.AluOpType.add,
            )
            # rank = sum_{jp<j} meq : mask in place, fold, reduce
            nc.vector.tensor_tensor(
                out=meq[:N],
                in0=meq[:N],
                in1=bass.AP(tensor=lt_full.tensor, offset=lt_full.offset,
                            ap=[[lt_full.ap[0][0], N], [N, N], [1, N]]),
                op=mybir.AluOpType.mult,
            )
            nc.vector.tensor_tensor(
                out=cf1[:N], in0=meq[:N, :, 0:HALF], in1=meq[:N, :, HALF:N],
                op=mybir.AluOpType.add,
            )
            nc.vector.tensor_tensor(
                out=cf2[:N], in0=cf1[:N, :, 0:HALF // 2], in1=cf1[:N, :, HALF // 2:HALF],
                op=mybir.AluOpType.add,
            )
            rank32 = cb.tile([NCH, N], F32, tag="rank32")
            nc.vector.tensor_reduce(
                out=rank32[:N], in_=cf2[:N], axis=mybir.AxisListType.X,
                op=mybir.AluOpType.add,
            )
            # scatter idx: g where first occurrence else -1; data: count
            fo = cb.tile([NCH, N], F16, tag="fo")
            nc.vector.tensor_scalar(
                out=fo[:N], in0=rank32[:N], scalar1=0.0, scalar2=None,
                op0=mybir.AluOpType.is_equal,
            )
            gp1 = cb.tile([NCH, N], F16, tag="gp1")
            nc.vector.tensor_scalar(
                out=gp1[:N], in0=gf[:], scalar1=1.0, scalar2=None,
                op0=mybir.AluOpType.add,
            )
            idxf = cb.tile([NCH, N], F16, tag="idxf")
            nc.vector.tensor_tensor(
                out=idxf[:N], in0=fo[:N], in1=gp1[:N], op=mybir.AluOpType.mult,
            )
            nc.vector.tensor_scalar(
                out=idxf[:N], in0=idxf[:N], scalar1=-1.0, scalar2=None,
                op0=mybir.AluOpType.add,
            )
            idx16 = cb.tile([NCH, N], I16, tag="idx16")
            cnt16 = cb.tile([NCH, N], F16, tag="cnt16")
            nc.vector.memset(idx16[:NCH, :], -1)
            nc.vector.memset(cnt16[:NCH, :], 0.0)
            nc.vector.tensor_copy(out=idx16[:N], in_=idxf[:N])
            nc.vector.tensor_copy(out=cnt16[:N], in_=cnt32[:N])
            cmat = cpool.tile([NCH, L], F16, tag="cmat")
            nc.gpsimd.local_scatter(
                out_ap=cmat[:],
                data_ap=cnt16[:NCH],
                idxs_ap=idx16[:NCH],
                channels=NCH,
                num_elems=L,
                num_idxs=N,
            )
            cmats.append(cmat)

        for b in range(BPC):
            gf = gb[b]
            cmat = cmats[b]
            # C^T tiles for this batch
            ct_sb = ctp.tile([128, LT_TILES, N], F32R, tag="ct")
            for lc in range(LT_TILES):
                cs = min(128, L - lc * 128)
                pt = pst.tile([128, N], F16, tag="ps")
                nc.tensor.transpose(
                    out=pt[:cs, :],
                    in_=cmat[:N, lc * 128:lc * 128 + cs],
                    identity=ident16[:N, :N],
                )
                nc.scalar.activation(
                    out=ct_sb[:cs, lc, :], in_=pt[:cs, :],
                    func=mybir.ActivationFunctionType.Copy,
                )
            # counts part of LIN
            for lc in range(LT_TILES):
                cs = min(128, L - lc * 128)
                for nch in range(2):
                    sl = slice(nch * 512, (nch + 1) * 512)
                    nc.tensor.matmul(
                        out=lin_ps[b][:, sl],
                        lhsT=ct_sb[:cs, lc, :],
                        rhs=bias_tiles[lc][:cs, sl],
                        start=False,
                        stop=(lc == LT_TILES - 1),
                    )
            lin_sb = linp.tile([N, D], F32R, tag=f"lin{b}")
            nc.vector.tensor_add(lin_sb[:], lin_ps[b][:], xb[b][:])

            # ---- attention for this batch ----
            rsl = slice(b * N, (b + 1) * N)
            psal = pssm.tile([N, N], F32, tag="ps")
            for dk in range(DT):
                nc.tensor.matmul(
                    out=psal[:],
                    lhsT=at_sb[:, dk, rsl],
                    rhs=bt_sb[:, dk, rsl],
                    start=(dk == 0),
                    stop=(dk == DT - 1),
                )
            alpha_sb = small.tile([N, N], F32R, tag="alpha")
            nc.scalar.activation(
                out=alpha_sb[:], in_=psal[:],
                func=mybir.ActivationFunctionType.Relu,
            )
            psgt = pst.tile([N, N], F16, tag="ps")
            nc.tensor.transpose(out=psgt[:], in_=gf[:], identity=ident16[:N, :N])
            adjt_sb = small.tile([N, N], F32R, tag="adjt")
            nc.vector.tensor_scalar(
                out=adjt_sb[:], in0=psgt[:], scalar1=0.0, scalar2=None,
                op0=mybir.AluOpType.not_equal,
            )
            psal2 = pssm.tile([N, N], F32, tag="ps")
            nc.tensor.matmul(
                out=psal2[:], lhsT=adjt_sb[:], rhs=alpha_sb[:],
                start=True, stop=True,
            )
            al2_sb = small.tile([N, N], F32, tag="al2")
            nc.scalar.activation(
                out=al2_sb[:], in_=psal2[:],
                func=mybir.ActivationFunctionType.Copy,
            )
            psal2t = pssm.tile([N, N], F32, tag="ps")
            nc.tensor.transpose(out=psal2t[:], in_=al2_sb[:], identity=ident32[:N, :N])
            negmx = small.tile([N, 1], F32, tag="negmx")
            nc.vector.tensor_reduce(
                out=negmx[:], in_=psal2t[:], axis=mybir.AxisListType.X,
                op=mybir.AluOpType.max, negate=True,
            )
            sm_sb = small.tile([N, N], F32, tag="smexp")
            ssum = small.tile([N, 1], F32, tag="ssum")
            nc.scalar.activation(
                out=sm_sb[:], in_=psal2t[:],
                func=mybir.ActivationFunctionType.Exp,
                bias=negmx[:], scale=1.0, accum_out=ssum[:],
            )
            rsum = small.tile([N, 1], F32, tag="rsum")
            nc.vector.reciprocal(out=rsum[:], in_=ssum[:])
            al3t_sb = small.tile([N, N], F32R, tag="al3t")
            nc.scalar.activation(
                out=al3t_sb[:], in_=sm_sb[:],
                func=mybir.ActivationFunctionType.Copy,
                scale=rsum[:],
            )
            pso = psout.tile([N, D], F32, tag="pslin")
            for nch in range(2):
                sl = slice(nch * 512, (nch + 1) * 512)
                nc.tensor.matmul(
                    out=pso[:, sl], lhsT=al3t_sb[:], rhs=lin_sb[:, sl],
                    start=True, stop=True,
                )
            o_sb = outp.tile([N, D], F32, tag="osb")
            nc.scalar.activation(
                out=o_sb[:], in_=pso[:],
                func=mybir.ActivationFunctionType.Copy,
            )
            nc.sync.dma_start(out=out_d.ap()[b * N:(b + 1) * N, :], in_=o_sb[:])


def _prep_inputs(feature, graph, W0, W1, bias, dp_Wa, dp_ba, dp_Wb, dp_bb):
    feature = np.ascontiguousarray(np.asarray(feature, dtype=np.float32))
    graph = np.asarray(graph)
    bias = np.ascontiguousarray(np.asarray(bias, dtype=np.float32))
    wct = np.ascontiguousarray(np.asarray(W0, np.float32).T
                               + np.asarray(W1, np.float32).T)
    wat = np.ascontiguousarray(np.asarray(dp_Wa, np.float32).T)
    wbt = np.ascontiguousarray(np.asarray(dp_Wb, np.float32).T)
    ba = np.ascontiguousarray(np.asarray(dp_ba, np.float32))
    bb = np.ascontiguousarray(np.asarray(dp_bb, np.float32))
    g16 = graph.astype(np.float16)  # labels < 2048: exact in fp16
    j = np.arange(N)
    ltmask = np.ascontiguousarray(
        (j[None, :] < j[:, None]).astype(np.float16).reshape(-1))
    id32 = np.eye(128, dtype=np.float32)
    id16 = np.eye(128, dtype=np.float16)

    in_maps = []
    for c in range(NCORES):
        bs = slice(c * BPC, (c + 1) * BPC)
        in_maps.append({
            "x": np.ascontiguousarray(feature[bs].reshape(R, D)),
            "id32": id32,
            "id16": id16,
            "g16": np.ascontiguousarray(g16[bs].reshape(R, N)),
            "ltmask": ltmask,
            "wct": wct,
            "wat": wat,
            "wbt": wbt,
            "bias": bias,
            "ba": ba,
            "bb": bb,
        })
    return in_maps


def get_program():
    if "nc" not in _CACHE:
        _CACHE["nc"] = _build_program()
    return _CACHE["nc"]


def kernel(feature, graph, W0, W1, bias, dp_Wa, dp_ba, dp_Wb, dp_bb,
           get_alpha=0, **_ignored):
    from concourse.bass_utils import run_bass_kernel_spmd

    nc = get_program()
    in_maps = _prep_inputs(feature, graph, W0, W1, bias, dp_Wa, dp_ba,
                           dp_Wb, dp_bb)
    res = run_bass_kernel_spmd(nc, in_maps, list(range(NCORES)))
    out = np.concatenate(
        [res.results[c]["out"].reshape(BPC, N, D) for c in range(NCORES)], axis=0
    )
    return out



# revision 12
# speedup vs baseline: 1.9721x; 1.9721x over previous
"""Trainium2 Bass kernel for CorrelatedGraphConv.

Reference computation (per batch b, N=100 rows, D=1024, L=2000 labels):
    adj   = (graph != 0)
    lin   = x + x@W0.T + x@W1.T + sum_j bias[graph[:, j]]
    a     = x@Wa.T + ba ; bvec = x@Wb.T + bb
    alpha = relu(a @ bvec.T)
    alpha = softmax(adj @ alpha, axis=0)   # over rows i
    out   = alpha @ lin

Strategy: data-parallel over batch across 8 cores (2 batches/core), all
matmuls in fp16 (fp32 PSUM accumulation).

Key reformulations vs the straightforward lowering:
  * a @ b.T == x@M@x.T + (x@v1) 1^T + 1 (x@v2 + c0)^T with M = Wa.T@Wb,
    v1 = Wa.T@bb, v2 = Wb.T@ba, c0 = ba.bb precomputed on host. This
    halves the projection matmul work and replaces 8 MB of fp32 weights
    with 2 MB of fp16. The rank-1 terms are two extra contraction-1
    matmuls accumulated into the same PSUM tile.
  * x + x@(W0.T+W1.T) == x@(W0.T+W1.T+I): the +x is folded into the
    host-precomputed combined weight matrix.
  * The label-gather sum_j bias[g[i,j]] is C @ bias where C[i,l] =
    #{j : g[i,j]==l}. Counts are computed per (row, occurrence) with a
    pairwise-equality reduce on DVE, then scattered at idx=g for EVERY
    occurrence: equal labels carry equal counts, so colliding scatter
    lanes write identical values and the result is order-independent.
    No first-occurrence masking is needed.
"""

import numpy as np

import concourse.bass as bass
import concourse.mybir as mybir
import concourse.tile as tile
from concourse import bacc, library_config

F32 = mybir.dt.float32
F16 = mybir.dt.float16
I16 = mybir.dt.int16

B, N, D, L = 16, 100, 1024, 2000
NCORES = 8
BPC = B // NCORES          # batches per core
R = BPC * N                # rows per core (200)
DT = D // 128              # 8 d-tiles
LT_TILES = (L + 127) // 128  # 16 label tiles (last is 80)
NCH = 112                  # scatter channels covering 100 rows
CHK = 4                    # histogram chunks per batch
AC = N // CHK              # rows-of-a per chunk (25)

_CACHE = {}


def _ap3(ap, dims, offset_elems=0):
    """Build a 3D AP view [(s0,c0),(s1,c1),(s2,c2)] over a 2D tile AP."""
    base = [list(d) for d in ap.ap]
    new = [base[0][:1] + [dims[0][1]] if False else [dims[0][0], dims[0][1]]
           for _ in range(1)]
    new = [[dims[0][0], dims[0][1]], [dims[1][0], dims[1][1]],
           [dims[2][0], dims[2][1]]]
    return bass.AP(tensor=ap.tensor, offset=ap.offset + offset_elems, ap=new)


def _build_program():
    nc = bacc.Bacc("TRN2", target_bir_lowering=False, debug=False,
                   num_devices=NCORES)

    x_d = nc.declare_dram_parameter("x16", [R, D], F16, isOutput=False)
    g_d = nc.declare_dram_parameter("g16", [R, N], F16, isOutput=False)
    gi_d = nc.declare_dram_parameter("gi16", [R, N], I16, isOutput=False)
    id16_d = nc.declare_dram_parameter("id16", [128, 128], F16, isOutput=False)
    id32_d = nc.declare_dram_parameter("id32", [128, 128], F32, isOutput=False)
    wc_d = nc.declare_dram_parameter("wc16", [D, D], F16, isOutput=False)
    m_d = nc.declare_dram_parameter("m16", [D, D], F16, isOutput=False)
    vv_d = nc.declare_dram_parameter("vv16", [128, DT * 2], F16, isOutput=False)
    c0_d = nc.declare_dram_parameter("c0col", [1, 1], F32, isOutput=False)
    bias_d = nc.declare_dram_parameter("bias16", [L, D], F16, isOutput=False)
    out_d = nc.declare_dram_parameter("out", [R, D], F32, isOutput=True)

    with tile.TileContext(nc) as tc:
        _emit(tc, x_d, g_d, gi_d, id16_d, id32_d, wc_d, m_d, vv_d, c0_d,
              bias_d, out_d)
    nc.compile()
    return nc


def _emit(tc, x_d, g_d, gi_d, id16_d, id32_d, wc_d, m_d, vv_d, c0_d,
          bias_d, out_d):
    nc = tc.nc
    import contextlib

    ctx = contextlib.ExitStack()
    with ctx:
        const = ctx.enter_context(tc.tile_pool(name="const", bufs=1))
        gpool = ctx.enter_context(tc.tile_pool(name="gtiles", bufs=1))
        xpool = ctx.enter_context(tc.tile_pool(name="xtiles", bufs=1))
        xtp = ctx.enter_context(tc.tile_pool(name="xt", bufs=1))
        ztp = ctx.enter_context(tc.tile_pool(name="zt", bufs=1))
        mstream = ctx.enter_context(tc.tile_pool(name="mstream", bufs=1))
        wstream = ctx.enter_context(tc.tile_pool(name="wstream", bufs=1))
        bstream = ctx.enter_context(tc.tile_pool(name="bstream", bufs=1))
        gmatp = ctx.enter_context(tc.tile_pool(name="gmat", bufs=1))
        eqp = ctx.enter_context(tc.tile_pool(name="eq", bufs=2))
        cfp = ctx.enter_context(tc.tile_pool(name="cf", bufs=2))
        scp = ctx.enter_context(tc.tile_pool(name="scat", bufs=1))
        cpool = ctx.enter_context(tc.tile_pool(name="cmat", bufs=1))
        ctp = ctx.enter_context(tc.tile_pool(name="ctmat", bufs=1))
        linp = ctx.enter_context(tc.tile_pool(name="lin", bufs=1))
        small = ctx.enter_context(tc.tile_pool(name="small", bufs=2))
        outp = ctx.enter_context(tc.tile_pool(name="outs", bufs=1))
        psw = ctx.enter_context(tc.tile_pool(name="psw", bufs=2, space="PSUM"))
        pszp = ctx.enter_context(tc.tile_pool(name="psz", bufs=2, space="PSUM"))
        pslin = ctx.enter_context(tc.tile_pool(name="pslin", bufs=2,
                                               space="PSUM"))

        nc.gpsimd.load_library(library_config.local_scatter)

        # ---------------- input DMAs ----------------
        idxb = []
        cntb = []
        for b in range(BPC):
            t = scp.tile([NCH, N], I16, tag=f"idx{b}")
            nc.vector.memset(t[:], -1)
            idxb.append(t)
            t2 = scp.tile([NCH, N], F16, tag=f"cnt{b}")
            nc.vector.memset(t2[:], 0.0)
            cntb.append(t2)
        gb = []
        for b in range(BPC):
            t = gpool.tile([N, N], F16, tag=f"gb{b}")
            nc.sync.dma_start(out=t[:], in_=g_d.ap()[b * N:(b + 1) * N, :])
            gb.append(t)
        for b in range(BPC):
            nc.sync.dma_start(out=idxb[b][:N], in_=gi_d.ap()[b * N:(b + 1) * N, :])
        ident16 = const.tile([128, 128], F16)
        nc.sync.dma_start(out=ident16[:], in_=id16_d.ap())
        xb = []
        for b in range(BPC):
            t = xpool.tile([N, D], F16, tag=f"xb{b}")
            nc.sync.dma_start(out=t[:], in_=x_d.ap()[b * N:(b + 1) * N, :])
            xb.append(t)
        vv_sb = const.tile([128, DT * 2], F16)
        nc.sync.dma_start(out=vv_sb[:], in_=vv_d.ap())
        c0_sb = const.tile([1, 1], F32)
        nc.sync.dma_start(out=c0_sb[:], in_=c0_d.ap())
        ident32 = const.tile([128, 128], F32)
        nc.sync.dma_start(out=ident32[:], in_=id32_d.ap())
        mp = []
        for dk in range(DT):
            t = mstream.tile([128, D], F16, tag=f"mp{dk}")
            nc.sync.dma_start(out=t[:], in_=m_d.ap()[dk * 128:(dk + 1) * 128, :])
            mp.append(t)
        wc = []
        for dk in range(DT):
            t = wstream.tile([128, D], F16, tag=f"wc{dk}")
            nc.sync.dma_start(out=t[:], in_=wc_d.ap()[dk * 128:(dk + 1) * 128, :])
            wc.append(t)
        bias_tiles = []
        for lc in range(LT_TILES):
            cs = min(128, L - lc * 128)
            t = bstream.tile([128, D], F16, tag=f"bt{lc}")
            nc.sync.dma_start(out=t[:cs], in_=bias_d.ap()[lc * 128:lc * 128 + cs, :])
            bias_tiles.append(t)

        # ---------------- small DVE inits ----------------
        ones_row = const.tile([1, N], F16)
        nc.vector.memset(ones_row[:], 1.0)

        # ---------------- PE: transposes of x and g ----------------
        xt_sb = xtp.tile([128, DT, R], F16)
        for b in range(BPC):
            for dk in range(DT):
                pt = psw.tile([128, N], F16, tag="ps")
                nc.tensor.transpose(
                    out=pt[:], in_=xb[b][:, dk * 128:(dk + 1) * 128],
                    identity=ident16[:N, :N],
                )
                nc.vector.tensor_copy(out=xt_sb[:, dk, b * N:(b + 1) * N],
                                      in_=pt[:])
        adjt = []
        for b in range(BPC):
            pg = psw.tile([N, N], F16, tag="ps")
            nc.tensor.transpose(out=pg[:], in_=gb[b][:], identity=ident16[:N, :N])
            at = small.tile([N, N], F16, tag=f"adjt{b}")
            nc.vector.tensor_scalar(
                out=at[:], in0=pg[:], scalar1=0.0, scalar2=None,
                op0=mybir.AluOpType.not_equal,
            )
            adjt.append(at)

        # ---------------- Scalar: replicate g rows for 2x-eligible eq ----
        # gmat[b][p, a, c] = g[b][p, a]  (a outer in chunks, c inner)
        gmats = []
        for b in range(BPC):
            gm = gmatp.tile([N, N, N], F16, tag=f"gmat{b}")
            gmats.append(gm)
        gchunks = []  # (b, c) -> emitted later interleaved
        # ---------------- DVE histogram chain, chunked ----------------
        # meq[p, a, c] = (g[p,c] == g[p,a]); cnt[p,a] = sum_c meq
        def emit_mat_chunk(b, c):
            gap = gb[b][:]
            (pstep, pcount), (fstep, fcount) = gap.ap[0], gap.ap[1]
            src = bass.AP(
                tensor=gap.tensor, offset=gap.offset + c * AC * fstep,
                ap=[[pstep, N], [fstep, AC], [0, N]],
            )
            nc.scalar.activation(
                out=gmats[b][:, c * AC:(c + 1) * AC, :], in_=src,
                func=mybir.ActivationFunctionType.Copy,
            )

        def emit_eq_chunk(b, c):
            gap = gb[b][:]
            (pstep, pcount), (fstep, fcount) = gap.ap[0], gap.ap[1]
            in0 = bass.AP(tensor=gap.tensor, offset=gap.offset,
                          ap=[[pstep, N], [0, AC], [fstep, N]])
            meq = eqp.tile([N, AC, N], F16, tag="meq")
            nc.vector.tensor_tensor(
                out=meq[:], in0=in0, in1=gmats[b][:, c * AC:(c + 1) * AC, :],
                op=mybir.AluOpType.is_equal,
            )
            cf1 = cfp.tile([N, AC, N // 2], F16, tag="cf1")
            nc.vector.tensor_tensor(
                out=cf1[:], in0=meq[:, :, 0:N // 2], in1=meq[:, :, N // 2:N],
                op=mybir.AluOpType.add,
            )
            cf2 = cfp.tile([N, AC, N // 4], F16, tag="cf2")
            nc.vector.tensor_tensor(
                out=cf2[:], in0=cf1[:, :, 0:N // 4], in1=cf1[:, :, N // 4:N // 2],
                op=mybir.AluOpType.add,
            )
            with nc.allow_low_precision(reason="counts <= 100 exact in fp16"):
                nc.vector.tensor_reduce(
                    out=cntb[b][0:N, c * AC:(c + 1) * AC], in_=cf2[:],
                    axis=mybir.AxisListType.X, op=mybir.AluOpType.add,
                )

        # interleave Scalar mat chunks and DVE eq chunks for batch 0
        for c in range(CHK):
            emit_mat_chunk(0, c)
            emit_eq_chunk(0, c)

        # ---------------- PE: z^T = M^T x^T ----------------
        zt_sb = ztp.tile([128, DT, R], F16)
        for dout in range(DT):
            psz = pszp.tile([128, R], F32, tag="pszh")
            for dk in range(DT):
                nc.tensor.matmul(
                    out=psz[:],
                    lhsT=mp[dk][:, dout * 128:(dout + 1) * 128],
                    rhs=xt_sb[:, dk, :],
                    start=(dk == 0), stop=(dk == DT - 1),
                )
            nc.scalar.activation(
                out=zt_sb[:, dout, :], in_=psz[:],
                func=mybir.ActivationFunctionType.Copy,
            )
        psxv1 = psw.tile([1, R], F32, tag="ps")
        for dk in range(DT):
            nc.tensor.matmul(
                out=psxv1[:], lhsT=vv_sb[:, dk * 2:dk * 2 + 1],
                rhs=xt_sb[:, dk, :],
                start=(dk == 0), stop=(dk == DT - 1),
            )
        psxv2 = psw.tile([1, R], F32, tag="ps")
        for dk in range(DT):
            nc.tensor.matmul(
                out=psxv2[:], lhsT=vv_sb[:, dk * 2 + 1:dk * 2 + 2],
                rhs=xt_sb[:, dk, :],
                start=(dk == 0), stop=(dk == DT - 1),
            )
        xv1_sb = small.tile([1, R], F16, tag="xv1")
        nc.scalar.activation(
            out=xv1_sb[:], in_=psxv1[:],
            func=mybir.ActivationFunctionType.Copy,
        )
        xv2_sb = small.tile([1, R], F16, tag="xv2")
        nc.scalar.activation(
            out=xv2_sb[:], in_=psxv2[:],
            func=mybir.ActivationFunctionType.Identity,
            bias=c0_sb[:], scale=1.0,
        )

        # ---------------- attention helpers ----------------
        alpha_sb = []
        al3t = []

        def emit_qk(b):
            rsl = slice(b * N, (b + 1) * N)
            psal = psw.tile([N, N], F32, tag="ps")
            for dk in range(DT):
                nc.tensor.matmul(
                    out=psal[:], lhsT=zt_sb[:, dk, rsl], rhs=xt_sb[:, dk, rsl],
                    start=(dk == 0), stop=False,
                )
            nc.tensor.matmul(
                out=psal[:], lhsT=xv1_sb[:, rsl], rhs=ones_row[:],
                start=False, stop=False,
            )
            nc.tensor.matmul(
                out=psal[:], lhsT=ones_row[:], rhs=xv2_sb[:, rsl],
                start=False, stop=True,
            )
            asb = small.tile([N, N], F16, tag=f"alpha{b}")
            nc.scalar.activation(
                out=asb[:], in_=psal[:],
                func=mybir.ActivationFunctionType.Relu,
            )
            alpha_sb.append(asb)

        def emit_softmax(b):
            psz2 = psw.tile([N, N], F32, tag="ps")
            nc.tensor.matmul(
                out=psz2[:], lhsT=adjt[b][:], rhs=alpha_sb[b][:],
                start=True, stop=True,
            )
            z2sb = small.tile([N, N], F32, tag="z2")
            nc.scalar.activation(
                out=z2sb[:], in_=psz2[:],
                func=mybir.ActivationFunctionType.Copy,
            )
            psz2t = psw.tile([N, N], F32, tag="ps")
            nc.tensor.transpose(out=psz2t[:], in_=z2sb[:],
                                identity=ident32[:N, :N])
            negmx = small.tile([N, 1], F32, tag="negmx")
            nc.vector.tensor_reduce(
                out=negmx[:], in_=psz2t[:], axis=mybir.AxisListType.X,
                op=mybir.AluOpType.max, negate=True,
            )
            sm_sb = small.tile([N, N], F32, tag="smexp")
            ssum = small.tile([N, 1], F32, tag="ssum")
            nc.scalar.activation(
                out=sm_sb[:], in_=psz2t[:],
                func=mybir.ActivationFunctionType.Exp,
                bias=negmx[:], scale=1.0, accum_out=ssum[:],
            )
            rsum = small.tile([N, 1], F32, tag="rsum")
            nc.vector.reciprocal(out=rsum[:], in_=ssum[:])
            a3 = small.tile([N, N], F16, tag=f"al3t{b}")
            nc.scalar.activation(
                out=a3[:], in_=sm_sb[:],
                func=mybir.ActivationFunctionType.Copy,
                scale=rsum[:],
            )
            al3t.append(a3)

        emit_qk(0)
        emit_softmax(0)

        # Scalar mat chunks + DVE eq chunks for batch 1
        for c in range(CHK):
            emit_mat_chunk(1, c)
            emit_eq_chunk(1, c)

        # ---------------- PE: lin = x @ (Wc + I) ----------------
        lin_ps = []
        for b in range(BPC):
            lp = pslin.tile([N, D], F32, tag="pslin")
            lin_ps.append(lp)
        for dk in range(DT):
            for b in range(BPC):
                for nch in range(2):
                    sl = slice(nch * 512, (nch + 1) * 512)
                    nc.tensor.matmul(
                        out=lin_ps[b][:, sl],
                        lhsT=xt_sb[:, dk, b * N:(b + 1) * N],
                        rhs=wc[dk][:, sl],
                        start=(dk == 0), stop=False,
                    )

        # ---------------- gpsimd: scatter counts ----------------
        cmats = []
        for b in range(BPC):
            cmat = cpool.tile([NCH, L], F16, tag=f"cmat{b}")
            nc.gpsimd.local_scatter(
                out_ap=cmat[:], data_ap=cntb[b][:NCH], idxs_ap=idxb[b][:NCH],
                channels=NCH, num_elems=L, num_idxs=N,
            )
            cmats.append(cmat)

        # ---------------- counts matmuls + av per batch ----------------
        for b in range(BPC):
            ct_sb = ctp.tile([128, LT_TILES, N], F16, tag=f"ct{b}")
            for lc in range(LT_TILES):
                cs = min(128, L - lc * 128)
                pt = psw.tile([128, N], F16, tag="ps")
                nc.tensor.transpose(
                    out=pt[:cs, :], in_=cmats[b][:N, lc * 128:lc * 128 + cs],
                    identity=ident16[:N, :N],
                )
                nc.scalar.activation(
                    out=ct_sb[:cs, lc, :], in_=pt[:cs, :],
                    func=mybir.ActivationFunctionType.Copy,
                )
            for lc in range(LT_TILES):
                cs = min(128, L - lc * 128)
                for nch in range(2):
                    sl = slice(nch * 512, (nch + 1) * 512)
                    nc.tensor.matmul(
                        out=lin_ps[b][:, sl],
                        lhsT=ct_sb[:cs, lc, :],
                        rhs=bias_tiles[lc][:cs, sl],
                        start=False, stop=(lc == LT_TILES - 1),
                    )
            lin_sb = linp.tile([N, D], F16, tag=f"lin{b}")
            nc.scalar.activation(
                out=lin_sb[:], in_=lin_ps[b][:],
                func=mybir.ActivationFunctionType.Copy,
            )
            if b + 1 < BPC:
                emit_qk(b + 1)
                emit_softmax(b + 1)
            pso = pslin.tile([N, D], F32, tag="pslin")
            for nch in range(2):
                sl = slice(nch * 512, (nch + 1) * 512)
                nc.tensor.matmul(
                    out=pso[:, sl], lhsT=al3t[b][:], rhs=lin_sb[:, sl],
                    start=True, stop=True,
                )
            o_sb = outp.tile([N, D], F32, tag=f"osb{b}")
            nc.scalar.activation(
                out=o_sb[:], in_=pso[:],
                func=mybir.ActivationFunctionType.Copy,
            )
            nc.sync.dma_start(out=out_d.ap()[b * N:(b + 1) * N, :], in_=o_sb[:])


def _prep_inputs(feature, graph, W0, W1, bias, dp_Wa, dp_ba, dp_Wb, dp_bb):
    feature = np.asarray(feature, dtype=np.float32)
    graph = np.asarray(graph)
    bias16 = np.ascontiguousarray(np.asarray(bias, np.float32).astype(np.float16))
    W0 = np.asarray(W0, np.float32)
    W1 = np.asarray(W1, np.float32)
    Wa = np.asarray(dp_Wa, np.float32)
    Wb = np.asarray(dp_Wb, np.float32)
    ba = np.asarray(dp_ba, np.float32)
    bb = np.asarray(dp_bb, np.float32)
    wc16 = np.ascontiguousarray(
        (W0.T + W1.T + np.eye(D, dtype=np.float32)).astype(np.float16))
    m16 = np.ascontiguousarray((Wa.T @ Wb).astype(np.float16))
    v1 = Wa.T @ bb
    v2 = Wb.T @ ba
    c0 = float(ba @ bb)
    vv16 = np.ascontiguousarray(
        np.stack([v1, v2], axis=1).reshape(DT, 128, 2)
        .transpose(1, 0, 2).reshape(128, DT * 2).astype(np.float16))
    c0col = np.array([[c0]], dtype=np.float32)
    x16 = feature.astype(np.float16)
    g16 = graph.astype(np.float16)   # labels < 2048: exact in fp16
    gi16 = graph.astype(np.int16)
    id16 = np.eye(128, dtype=np.float16)
    id32 = np.eye(128, dtype=np.float32)

    in_maps = []
    for c in range(NCORES):
        bs = slice(c * BPC, (c + 1) * BPC)
        in_maps.append({
            "x16": np.ascontiguousarray(x16[bs].reshape(R, D)),
            "g16": np.ascontiguousarray(g16[bs].reshape(R, N)),
            "gi16": np.ascontiguousarray(gi16[bs].reshape(R, N)),
            "id16": id16,
            "id32": id32,
            "wc16": wc16,
            "m16": m16,
            "vv16": vv16,
            "c0col": c0col,
            "bias16": bias16,
        })
    return in_maps


def get_program():
    if "nc" not in _CACHE:
        _CACHE["nc"] = _build_program()
    return _CACHE["nc"]


def kernel(feature, graph, W0, W1, bias, dp_Wa, dp_ba, dp_Wb, dp_bb,
           get_alpha=0, **_ignored):
    from concourse.bass_utils import run_bass_kernel_spmd

    nc = get_program()
    in_maps = _prep_inputs(feature, graph, W0, W1, bias, dp_Wa, dp_ba,
                           dp_Wb, dp_bb)
    res = run_bass_kernel_spmd(nc, in_maps, list(range(NCORES)))
    out = np.concatenate(
        [res.results[c]["out"].reshape(BPC, N, D) for c in range(NCORES)], axis=0
    )
    return out


# revision 15
# speedup vs baseline: 2.0129x; 1.0207x over previous
"""Trainium2 Bass kernel for CorrelatedGraphConv.

Reference computation (per batch b, N=100 rows, D=1024, L=2000 labels):
    adj   = (graph != 0)
    lin   = x + x@W0.T + x@W1.T + sum_j bias[graph[:, j]]
    a     = x@Wa.T + ba ; bvec = x@Wb.T + bb
    alpha = relu(a @ bvec.T)
    alpha = softmax(adj @ alpha, axis=0)   # over rows i
    out   = alpha @ lin

Strategy: data-parallel over batch across 8 cores (2 batches/core), all
matmuls in fp16 (fp32 PSUM accumulation).

Key reformulations vs the straightforward lowering:
  * a @ b.T == x@M@x.T + (x@v1) 1^T + 1 (x@v2 + c0)^T with M = Wa.T@Wb,
    v1 = Wa.T@bb, v2 = Wb.T@ba, c0 = ba.bb precomputed on host. This
    halves the projection matmul work and replaces 8 MB of fp32 weights
    with 2 MB of fp16. The rank-1 terms are two extra contraction-1
    matmuls accumulated into the same PSUM tile.
  * x + x@(W0.T+W1.T) == x@(W0.T+W1.T+I): the +x is folded into the
    host-precomputed combined weight matrix.
  * The label-gather sum_j bias[g[i,j]] is C @ bias where C[i,l] =
    #{j : g[i,j]==l}. Counts are computed per (row, occurrence) with a
    pairwise-equality reduce on DVE, then scattered at idx=g for EVERY
    occurrence: equal labels carry equal counts, so colliding scatter
    lanes write identical values and the result is order-independent.
    No first-occurrence masking is needed.
"""

import numpy as np

import concourse.bass as bass
import concourse.mybir as mybir
import concourse.tile as tile
from concourse import bacc, library_config

F32 = mybir.dt.float32
F16 = mybir.dt.float16
I16 = mybir.dt.int16

B, N, D, L = 16, 100, 1024, 2000
NCORES = 8
BPC = B // NCORES          # batches per core
R = BPC * N                # rows per core (200)
DT = D // 128              # 8 d-tiles
LT_TILES = (L + 127) // 128  # 16 label tiles (last is 80)
NCH = 112                  # scatter channels covering 100 rows
CHK = 4                    # histogram chunks per batch
AC = N // CHK              # rows-of-a per chunk (25)

_CACHE = {}


def _ap3(ap, dims, offset_elems=0):
    """Build a 3D AP view [(s0,c0),(s1,c1),(s2,c2)] over a 2D tile AP."""
    base = [list(d) for d in ap.ap]
    new = [base[0][:1] + [dims[0][1]] if False else [dims[0][0], dims[0][1]]
           for _ in range(1)]
    new = [[dims[0][0], dims[0][1]], [dims[1][0], dims[1][1]],
           [dims[2][0], dims[2][1]]]
    return bass.AP(tensor=ap.tensor, offset=ap.offset + offset_elems, ap=new)


def _build_program():
    nc = bacc.Bacc("TRN2", target_bir_lowering=False, debug=False,
                   num_devices=NCORES)

    x_d = nc.declare_dram_parameter("x16", [100, BPC * D], F16, isOutput=False)
    g_d = nc.declare_dram_parameter("g16", [100, BPC * N], F16, isOutput=False)
    gi_d = nc.declare_dram_parameter("gi16", [R, N], I16, isOutput=False)
    id16_d = nc.declare_dram_parameter("id16", [128, 128], F16, isOutput=False)
    id32_d = nc.declare_dram_parameter("id32", [128, 128], F32, isOutput=False)
    wc_d = nc.declare_dram_parameter("wc16", [128, DT * D], F16, isOutput=False)
    m_d = nc.declare_dram_parameter("m16", [128, DT * D], F16, isOutput=False)
    vv_d = nc.declare_dram_parameter("vv16", [128, DT * 2], F16, isOutput=False)
    c0_d = nc.declare_dram_parameter("c0col", [1, 1], F32, isOutput=False)
    bias_d = nc.declare_dram_parameter("bias16", [128, LT_TILES * D], F16,
                                       isOutput=False)
    out_d = nc.declare_dram_parameter("out", [R, D], F32, isOutput=True)

    with tile.TileContext(nc) as tc:
        _emit(tc, x_d, g_d, gi_d, id16_d, id32_d, wc_d, m_d, vv_d, c0_d,
              bias_d, out_d)
    nc.compile()
    return nc


def _emit(tc, x_d, g_d, gi_d, id16_d, id32_d, wc_d, m_d, vv_d, c0_d,
          bias_d, out_d):
    nc = tc.nc
    import contextlib

    ctx = contextlib.ExitStack()
    with ctx:
        const = ctx.enter_context(tc.tile_pool(name="const", bufs=1))
        gpool = ctx.enter_context(tc.tile_pool(name="gtiles", bufs=1))
        xpool = ctx.enter_context(tc.tile_pool(name="xtiles", bufs=1))
        xtp = ctx.enter_context(tc.tile_pool(name="xt", bufs=1))
        ztp = ctx.enter_context(tc.tile_pool(name="zt", bufs=1))
        mstream = ctx.enter_context(tc.tile_pool(name="mstream", bufs=1))
        wstream = ctx.enter_context(tc.tile_pool(name="wstream", bufs=1))
        bstream = ctx.enter_context(tc.tile_pool(name="bstream", bufs=1))
        gmatp = ctx.enter_context(tc.tile_pool(name="gmat", bufs=1))
        eqp = ctx.enter_context(tc.tile_pool(name="eq", bufs=2))
        cfp = ctx.enter_context(tc.tile_pool(name="cf", bufs=2))
        scp = ctx.enter_context(tc.tile_pool(name="scat", bufs=1))
        cpool = ctx.enter_context(tc.tile_pool(name="cmat", bufs=1))
        ctp = ctx.enter_context(tc.tile_pool(name="ctmat", bufs=1))
        linp = ctx.enter_context(tc.tile_pool(name="lin", bufs=1))
        small = ctx.enter_context(tc.tile_pool(name="small", bufs=2))
        outp = ctx.enter_context(tc.tile_pool(name="outs", bufs=1))
        psw = ctx.enter_context(tc.tile_pool(name="psw", bufs=2, space="PSUM"))
        pszp = ctx.enter_context(tc.tile_pool(name="psz", bufs=2, space="PSUM"))
        pslin = ctx.enter_context(tc.tile_pool(name="pslin", bufs=2,
                                               space="PSUM"))

        nc.gpsimd.load_library(library_config.local_scatter)

        # -------- input DMAs (consolidated: ~12 descriptors) --------
        idxb = []
        cntb = []
        for b in range(BPC):
            t = scp.tile([NCH, N], I16, tag=f"idx{b}")
            nc.vector.memset(t[:], -1)
            idxb.append(t)
            t2 = scp.tile([NCH, N], F16, tag=f"cnt{b}")
            nc.vector.memset(t2[:], 0.0)
            cntb.append(t2)
        gall = gpool.tile([N, BPC, N], F16, tag="gall")
        nc.sync.dma_start(out=gall[:], in_=g_d.ap())
        gb = [gall[:, b, :] for b in range(BPC)]
        for b in range(BPC):
            nc.sync.dma_start(out=idxb[b][:N], in_=gi_d.ap()[b * N:(b + 1) * N, :])
        ident16 = const.tile([128, 128], F16)
        nc.sync.dma_start(out=ident16[:], in_=id16_d.ap())
        xall = xpool.tile([N, BPC, D], F16, tag="xall")
        nc.sync.dma_start(out=xall[:], in_=x_d.ap())
        xb = [xall[:, b, :] for b in range(BPC)]
        vv_sb = const.tile([128, DT * 2], F16)
        nc.sync.dma_start(out=vv_sb[:], in_=vv_d.ap())
        c0_sb = const.tile([1, 1], F32)
        nc.sync.dma_start(out=c0_sb[:], in_=c0_d.ap())
        ident32 = const.tile([128, 128], F32)
        nc.sync.dma_start(out=ident32[:], in_=id32_d.ap())
        mall = mstream.tile([128, DT, D], F16, tag="mall")
        nc.sync.dma_start(out=mall[:], in_=m_d.ap())
        mp = [mall[:, dk, :] for dk in range(DT)]
        wcall = wstream.tile([128, DT, D], F16, tag="wcall")
        nc.sync.dma_start(out=wcall[:], in_=wc_d.ap())
        wc = [wcall[:, dk, :] for dk in range(DT)]
        ball = bstream.tile([128, LT_TILES, D], F16, tag="ball")
        for q in range(4):
            nc.sync.dma_start(
                out=ball[:, q * 4:(q + 1) * 4, :],
                in_=bias_d.ap()[:, q * 4 * D:(q + 1) * 4 * D],
            )
        bias_tiles = [ball[:, lc, :] for lc in range(LT_TILES)]

        # ---------------- small DVE inits ----------------
        ones_row = const.tile([1, N], F16)
        nc.vector.memset(ones_row[:], 1.0)

        # ---------------- PE: transposes of x and g ----------------
        xt_sb = xtp.tile([128, DT, R], F16)
        for b in range(BPC):
            for dk in range(DT):
                pt = psw.tile([128, N], F16, tag="ps")
                nc.tensor.transpose(
                    out=pt[:], in_=xb[b][:, dk * 128:(dk + 1) * 128],
                    identity=ident16[:N, :N],
                )
                nc.vector.tensor_copy(out=xt_sb[:, dk, b * N:(b + 1) * N],
                                      in_=pt[:])
        adjt = []
        for b in range(BPC):
            pg = psw.tile([N, N], F16, tag="ps")
            nc.tensor.transpose(out=pg[:], in_=gb[b], identity=ident16[:N, :N])
            at = small.tile([N, N], F16, tag=f"adjt{b}")
            nc.vector.tensor_scalar(
                out=at[:], in0=pg[:], scalar1=0.0, scalar2=None,
                op0=mybir.AluOpType.not_equal,
            )
            adjt.append(at)

        # ---------------- Scalar: replicate g rows for 2x-eligible eq ----
        # gmat[b][p, a, c] = g[b][p, a]  (a outer in chunks, c inner)
        gmats = []
        for b in range(BPC):
            gm = gmatp.tile([N, N, N], F16, tag=f"gmat{b}")
            gmats.append(gm)
        gchunks = []  # (b, c) -> emitted later interleaved
        # ---------------- DVE histogram chain, chunked ----------------
        # meq[p, a, c] = (g[p,c] == g[p,a]); cnt[p,a] = sum_c meq
        def emit_mat_chunk(b, c):
            gap = gb[b]
            (pstep, pcount), (fstep, fcount) = gap.ap[0], gap.ap[1]
            src = bass.AP(
                tensor=gap.tensor, offset=gap.offset + c * AC * fstep,
                ap=[[pstep, N], [fstep, AC], [0, N]],
            )
            nc.scalar.activation(
                out=gmats[b][:, c * AC:(c + 1) * AC, :], in_=src,
                func=mybir.ActivationFunctionType.Copy,
            )

        def emit_eq_chunk(b, c):
            gap = gb[b]
            (pstep, pcount), (fstep, fcount) = gap.ap[0], gap.ap[1]
            in0 = bass.AP(tensor=gap.tensor, offset=gap.offset,
                          ap=[[pstep, N], [0, AC], [fstep, N]])
            meq = eqp.tile([N, AC, N], F16, tag="meq")
            nc.vector.tensor_tensor(
                out=meq[:], in0=in0, in1=gmats[b][:, c * AC:(c + 1) * AC, :],
                op=mybir.AluOpType.is_equal,
            )
            cf1 = cfp.tile([N, AC, N // 2], F16, tag="cf1")
            nc.vector.tensor_tensor(
                out=cf1[:], in0=meq[:, :, 0:N // 2], in1=meq[:, :, N // 2:N],
                op=mybir.AluOpType.add,
            )
            cf2 = cfp.tile([N, AC, N // 4], F16, tag="cf2")
            nc.vector.tensor_tensor(
                out=cf2[:], in0=cf1[:, :, 0:N // 4], in1=cf1[:, :, N // 4:N // 2],
                op=mybir.AluOpType.add,
            )
            with nc.allow_low_precision(reason="counts <= 100 exact in fp16"):
                nc.vector.tensor_reduce(
                    out=cntb[b][0:N, c * AC:(c + 1) * AC], in_=cf2[:],
                    axis=mybir.AxisListType.X, op=mybir.AluOpType.add,
                )

        # interleave Scalar mat chunks and DVE eq chunks for batch 0
        for c in range(CHK):
            emit_mat_chunk(0, c)
            emit_eq_chunk(0, c)

        # ---------------- PE: z^T = M^T x^T ----------------
        zt_sb = ztp.tile([128, DT, R], F16)
        for dout in range(DT):
            psz = pszp.tile([128, R], F32, tag="pszh")
            for dk in range(DT):
                nc.tensor.matmul(
                    out=psz[:],
                    lhsT=mp[dk][:, dout * 128:(dout + 1) * 128],
                    rhs=xt_sb[:, dk, :],
                    start=(dk == 0), stop=(dk == DT - 1),
                )
            nc.scalar.activation(
                out=zt_sb[:, dout, :], in_=psz[:],
                func=mybir.ActivationFunctionType.Copy,
            )
        psxv1 = psw.tile([1, R], F32, tag="ps")
        for dk in range(DT):
            nc.tensor.matmul(
                out=psxv1[:], lhsT=vv_sb[:, dk * 2:dk * 2 + 1],
                rhs=xt_sb[:, dk, :],
                start=(dk == 0), stop=(dk == DT - 1),
            )
        psxv2 = psw.tile([1, R], F32, tag="ps")
        for dk in range(DT):
            nc.tensor.matmul(
                out=psxv2[:], lhsT=vv_sb[:, dk * 2 + 1:dk * 2 + 2],
                rhs=xt_sb[:, dk, :],
                start=(dk == 0), stop=(dk == DT - 1),
            )
        xv1_sb = small.tile([1, R], F16, tag="xv1")
        nc.scalar.activation(
            out=xv1_sb[:], in_=psxv1[:],
            func=mybir.ActivationFunctionType.Copy,
        )
        xv2_sb = small.tile([1, R], F16, tag="xv2")
        nc.scalar.activation(
            out=xv2_sb[:], in_=psxv2[:],
            func=mybir.ActivationFunctionType.Identity,
            bias=c0_sb[:], scale=1.0,
        )

        # ---------------- attention helpers ----------------
        alpha_sb = []
        al3t = []

        def emit_qk(b):
            rsl = slice(b * N, (b + 1) * N)
            psal = psw.tile([N, N], F32, tag="ps")
            for dk in range(DT):
                nc.tensor.matmul(
                    out=psal[:], lhsT=zt_sb[:, dk, rsl], rhs=xt_sb[:, dk, rsl],
                    start=(dk == 0), stop=False,
                )
            nc.tensor.matmul(
                out=psal[:], lhsT=xv1_sb[:, rsl], rhs=ones_row[:],
                start=False, stop=False,
            )
            nc.tensor.matmul(
                out=psal[:], lhsT=ones_row[:], rhs=xv2_sb[:, rsl],
                start=False, stop=True,
            )
            asb = small.tile([N, N], F16, tag=f"alpha{b}")
            nc.scalar.activation(
                out=asb[:], in_=psal[:],
                func=mybir.ActivationFunctionType.Relu,
            )
            alpha_sb.append(asb)

        def emit_softmax(b):
            psz2 = psw.tile([N, N], F32, tag="ps")
            nc.tensor.matmul(
                out=psz2[:], lhsT=adjt[b][:], rhs=alpha_sb[b][:],
                start=True, stop=True,
            )
            z2sb = small.tile([N, N], F32, tag="z2")
            nc.scalar.activation(
                out=z2sb[:], in_=psz2[:],
                func=mybir.ActivationFunctionType.Copy,
            )
            psz2t = psw.tile([N, N], F32, tag="ps")
            nc.tensor.transpose(out=psz2t[:], in_=z2sb[:],
                                identity=ident32[:N, :N])
            negmx = small.tile([N, 1], F32, tag="negmx")
            nc.vector.tensor_reduce(
                out=negmx[:], in_=psz2t[:], axis=mybir.AxisListType.X,
                op=mybir.AluOpType.max, negate=True,
            )
            sm_sb = small.tile([N, N], F32, tag="smexp")
            ssum = small.tile([N, 1], F32, tag="ssum")
            nc.scalar.activation(
                out=sm_sb[:], in_=psz2t[:],
                func=mybir.ActivationFunctionType.Exp,
                bias=negmx[:], scale=1.0, accum_out=ssum[:],
            )
            rsum = small.tile([N, 1], F32, tag="rsum")
            nc.vector.reciprocal(out=rsum[:], in_=ssum[:])
            a3 = small.tile([N, N], F16, tag=f"al3t{b}")
            nc.scalar.activation(
                out=a3[:], in_=sm_sb[:],
                func=mybir.ActivationFunctionType.Copy,
                scale=rsum[:],
            )
            al3t.append(a3)

        emit_qk(0)
        emit_softmax(0)

        # Scalar mat chunks + DVE eq chunks for batch 1
        for c in range(CHK):
            emit_mat_chunk(1, c)
            emit_eq_chunk(1, c)

        # ---------------- PE: lin = x @ (Wc + I) ----------------
        lin_ps = []
        for b in range(BPC):
            lp = pslin.tile([N, D], F32, tag="pslin")
            lin_ps.append(lp)
        for dk in range(DT):
            for b in range(BPC):
                for nch in range(2):
                    sl = slice(nch * 512, (nch + 1) * 512)
                    nc.tensor.matmul(
                        out=lin_ps[b][:, sl],
                        lhsT=xt_sb[:, dk, b * N:(b + 1) * N],
                        rhs=wc[dk][:, sl],
                        start=(dk == 0), stop=False,
                    )

        # ---------------- gpsimd: scatter counts ----------------
        cmats = []
        for b in range(BPC):
            cmat = cpool.tile([NCH, L], F16, tag=f"cmat{b}")
            nc.gpsimd.local_scatter(
                out_ap=cmat[:], data_ap=cntb[b][:NCH], idxs_ap=idxb[b][:NCH],
                channels=NCH, num_elems=L, num_idxs=N,
            )
            cmats.append(cmat)

        # ---------------- counts matmuls + av per batch ----------------
        for b in range(BPC):
            ct_sb = ctp.tile([128, LT_TILES, N], F16, tag=f"ct{b}")
            for lc in range(LT_TILES):
                cs = min(128, L - lc * 128)
                pt = psw.tile([128, N], F16, tag="ps")
                nc.tensor.transpose(
                    out=pt[:cs, :], in_=cmats[b][:N, lc * 128:lc * 128 + cs],
                    identity=ident16[:N, :N],
                )
                nc.scalar.activation(
                    out=ct_sb[:cs, lc, :], in_=pt[:cs, :],
                    func=mybir.ActivationFunctionType.Copy,
                )
            for lc in range(LT_TILES):
                cs = min(128, L - lc * 128)
                for nch in range(2):
                    sl = slice(nch * 512, (nch + 1) * 512)
                    nc.tensor.matmul(
                        out=lin_ps[b][:, sl],
                        lhsT=ct_sb[:cs, lc, :],
                        rhs=bias_tiles[lc][:cs, sl],
                        start=False, stop=(lc == LT_TILES - 1),
                    )
            lin_sb = linp.tile([N, D], F16, tag=f"lin{b}")
            nc.scalar.activation(
                out=lin_sb[:], in_=lin_ps[b][:],
                func=mybir.ActivationFunctionType.Copy,
            )
            if b + 1 < BPC:
                emit_qk(b + 1)
                emit_softmax(b + 1)
            pso = pslin.tile([N, D], F32, tag="pslin")
            for nch in range(2):
                sl = slice(nch * 512, (nch + 1) * 512)
                nc.tensor.matmul(
                    out=pso[:, sl], lhsT=al3t[b][:], rhs=lin_sb[:, sl],
                    start=True, stop=True,
                )
            o_sb = outp.tile([N, D], F32, tag=f"osb{b}")
            nc.scalar.activation(
                out=o_sb[:], in_=pso[:],
                func=mybir.ActivationFunctionType.Copy,
            )
            nc.sync.dma_start(out=out_d.ap()[b * N:(b + 1) * N, :], in_=o_sb[:])


def _prep_inputs(feature, graph, W0, W1, bias, dp_Wa, dp_ba, dp_Wb, dp_bb):
    feature = np.asarray(feature, dtype=np.float32)
    graph = np.asarray(graph)
    bias16 = np.ascontiguousarray(np.asarray(bias, np.float32).astype(np.float16))
    W0 = np.asarray(W0, np.float32)
    W1 = np.asarray(W1, np.float32)
    Wa = np.asarray(dp_Wa, np.float32)
    Wb = np.asarray(dp_Wb, np.float32)
    ba = np.asarray(dp_ba, np.float32)
    bb = np.asarray(dp_bb, np.float32)
    wc16 = np.ascontiguousarray(
        (W0.T + W1.T + np.eye(D, dtype=np.float32)).astype(np.float16))
    m16 = np.ascontiguousarray((Wa.T @ Wb).astype(np.float16))
    v1 = Wa.T @ bb
    v2 = Wb.T @ ba
    c0 = float(ba @ bb)
    vv16 = np.ascontiguousarray(
        np.stack([v1, v2], axis=1).reshape(DT, 128, 2)
        .transpose(1, 0, 2).reshape(128, DT * 2).astype(np.float16))
    c0col = np.array([[c0]], dtype=np.float32)
    x16 = feature.astype(np.float16)
    g16 = graph.astype(np.float16)   # labels < 2048: exact in fp16
    gi16 = graph.astype(np.int16)
    id16 = np.eye(128, dtype=np.float16)
    id32 = np.eye(128, dtype=np.float32)
    # interleaved [p, dk, :] panel layouts for single-descriptor DMAs
    wc16 = np.ascontiguousarray(
        wc16.reshape(DT, 128, D).transpose(1, 0, 2).reshape(128, DT * D))
    m16 = np.ascontiguousarray(
        m16.reshape(DT, 128, D).transpose(1, 0, 2).reshape(128, DT * D))
    bias_pad = np.zeros((LT_TILES * 128, D), np.float16)
    bias_pad[:L] = bias16
    bias16 = np.ascontiguousarray(
        bias_pad.reshape(LT_TILES, 128, D).transpose(1, 0, 2)
        .reshape(128, LT_TILES * D))

    in_maps = []
    for c in range(NCORES):
        bs = slice(c * BPC, (c + 1) * BPC)
        in_maps.append({
            "x16": np.ascontiguousarray(
                x16[bs].transpose(1, 0, 2).reshape(100, BPC * D)),
            "g16": np.ascontiguousarray(
                g16[bs].transpose(1, 0, 2).reshape(100, BPC * N)),
            "gi16": np.ascontiguousarray(gi16[bs].reshape(R, N)),
            "id16": id16,
            "id32": id32,
            "wc16": wc16,
            "m16": m16,
            "vv16": vv16,
            "c0col": c0col,
            "bias16": bias16,
        })
    return in_maps


def get_program():
    if "nc" not in _CACHE:
        _CACHE["nc"] = _build_program()
    return _CACHE["nc"]


def kernel(feature, graph, W0, W1, bias, dp_Wa, dp_ba, dp_Wb, dp_bb,
           get_alpha=0, **_ignored):
    from concourse.bass_utils import run_bass_kernel_spmd

    nc = get_program()
    in_maps = _prep_inputs(feature, graph, W0, W1, bias, dp_Wa, dp_ba,
                           dp_Wb, dp_bb)
    res = run_bass_kernel_spmd(nc, in_maps, list(range(NCORES)))
    out = np.concatenate(
        [res.results[c]["out"].reshape(BPC, N, D) for c in range(NCORES)], axis=0
    )
    return out
